# revision 4
# baseline (speedup 1.0000x reference)
"""Trainium2 Bass kernel for the NeuralBridgeSDE sampling problem.

Reference scan step s (column-vector convention, a = G G^T, c_s = 1/(T-t_s+EPS)):
    y   = vt - x
    h1  = tanh(W1c x + b1 + t_s W1[0])            W1c = W1[1:].T
    h2  = tanh(W2c h1 + b2)
    v   = W3c h2 + b3
    x'  = M_s x + A h2 + G dWt_s + k_s            (dWt pre-scaled by sqrt(dt))
          M_s = (1-BETA dt) I - dt c_s a,  A = dt G W3.T,
          k_s = dt BETA mu + dt c_s a vt + dt G b3
    ll += y . (Q_s x + gcol_s)
          Q_s = -BETA dt c_s I + 0.5 dt c_s^2 a
          gcol_s = c_s dt BETA mu - 0.5 dt c_s^2 (a vt)

Device layout: feature-major [feat, batch] tiles, batch 1024 split 128 per
core across 8 cores (pure data parallel; the scan is sequential in time).
The per-step critical cycle is only

    matmul(WA @ h2) -> tanh -> matmul(W2) -> tanh

because the first-layer pre-activation of step s+1 is computed directly:
    h1pre_{s+1} = (alpha W1c) x_s + (W1c G) dWt_s + (W1c a) xs2_s
                  + (W1c A) h2_s + [W1c k_s + b1 + t_{s+1} W1[0]]
with xs2_s = (-dt c_s) x_s and the bracket folded into the tanh bias.
Everything else (x', Q x, v, ll) runs off-chain on PE/DVE/ACT slack.
"""

import numpy as np

import concourse.bass as bass
import concourse.bacc as bacc
import concourse.tile as tile
from concourse import mybir
from concourse import bass_utils

BETA = 0.5
EPS = 1e-4
NS = 500
DX = 32
H = 128
NCORES = 8
BC = 128          # batch per core
PF = 4            # DMA prefetch depth (steps)
F32 = mybir.dt.float32


# ----------------------------------------------------------------- host math
def _host_tables(ts, W1, b1, W2, b2, W3, b3, Gmat, mu, v_target, ns=NS):
    ts = np.asarray(ts, np.float32)
    T = np.float32(ts[-1])
    dts = (ts[1:] - ts[:-1]).astype(np.float32)
    t_seq = np.empty(ns + 1, np.float32)
    t_seq[0] = ts[0]
    for s in range(ns):
        t_seq[s + 1] = np.float32(t_seq[s] + dts[s])
    D = ((T - t_seq[:ns]) + np.float32(EPS)).astype(np.float32)

    f64 = np.float64
    G = np.asarray(Gmat, f64)
    a = G @ G.T
    W1_ = np.asarray(W1, f64)
    W3c = np.asarray(W3, f64).T
    mu64 = np.asarray(mu, f64)
    vt64 = np.asarray(v_target, f64)
    avt = a @ vt64
    Gb3 = G @ np.asarray(b3, f64)

    dt64 = dts.astype(f64)
    c = 1.0 / D.astype(f64)
    dt0 = dt64.mean()
    alpha = 1.0 - BETA * dt0

    I = np.eye(DX)
    M = alpha * I[None] - (dt64 * c)[:, None, None] * a[None]       # [ns,32,32]
    Q = (-BETA * dt64 * c)[:, None, None] * I[None] + (
        0.5 * dt64 * c * c
    )[:, None, None] * a[None]                                      # [ns,32,32]
    # [M_s; G^T] stacked along K for the x' matmul over state=[x; dWt]
    LTAB2 = np.concatenate(
        [M, np.broadcast_to(G.T[None], (ns, DX, DX))], axis=1
    ).astype(np.float32)                                            # [ns,64,32]

    kvec = (dt64[:, None] * BETA * mu64[None]
            + (dt64 * c)[:, None] * avt[None]
            + dt64[:, None] * Gb3[None])                            # [ns,32]
    gcol = ((dt64 * c)[:, None] * BETA * mu64[None]
            - (0.5 * dt64 * c * c)[:, None] * avt[None])            # [ns,32]

    W1c = W1_[1:, :].T
    b1c = np.asarray(b1, f64)[None] + t_seq[:ns, None].astype(f64) * W1_[0][None]
    b1tot = b1c.copy()
    b1tot[1:] += kvec[: ns - 1] @ W1c.T                             # [ns,H]

    A = dt0 * (G @ W3c)                                             # [32,H]
    C64 = np.concatenate(
        [alpha * W1_[1:, :], G.T @ W1_[1:, :]], axis=0
    ).astype(np.float32)                                            # [64,H]
    return dict(
        LTAB=np.ascontiguousarray(LTAB2),
        QTAB=np.ascontiguousarray(Q.astype(np.float32)),            # [ns,32,32]
        KROW=np.ascontiguousarray(kvec.astype(np.float32)[None]),   # [1,ns,32]
        GCOLT=np.ascontiguousarray(gcol.astype(np.float32).T),      # [32,ns]
        B1T=np.ascontiguousarray(b1tot.astype(np.float32).T),       # [H,ns]
        B2COL=np.ascontiguousarray(np.asarray(b2, np.float32)[:, None]),
        C64=np.ascontiguousarray(C64),
        AW1=np.ascontiguousarray((a @ W1_[1:, :]).astype(np.float32)),   # [32,H]
        WA=np.ascontiguousarray((A.T @ W1_[1:, :]).astype(np.float32)),  # [H,H]
        W2L=np.ascontiguousarray(np.asarray(W2, np.float32)),
        VA=np.ascontiguousarray(A.T.astype(np.float32)),            # [H,32]
        W3L=np.ascontiguousarray(np.asarray(W3, np.float32)),       # [H,32]
        W1E=np.ascontiguousarray(np.asarray(W1, np.float32)[1:, :]),
        B3ROW=np.ascontiguousarray(np.asarray(b3, np.float32)[None]),
        VTCOL=np.ascontiguousarray(np.asarray(v_target, np.float32)[:, None]),
        sc2=(-(dt64 * c)).astype(np.float32),                       # [ns]
        sqdt=np.sqrt(dts).astype(np.float32),
    )


# ------------------------------------------------------------ device program
def _build_program(ns, sc2):
    nc = bacc.Bacc("TRN2", target_bir_lowering=False, debug=False,
                   num_devices=NCORES)
    f = F32
    t_in = {}
    for name, shape in [
        ("X0T", [DX, BC]), ("DWT", [ns, DX, BC]), ("LTAB", [ns, 64, DX]),
        ("QTAB", [ns, DX, DX]), ("KROW", [1, ns, DX]), ("GCOLT", [DX, ns]),
        ("B1T", [H, ns]), ("B2COL", [H, 1]), ("C64", [64, H]),
        ("AW1", [DX, H]), ("WA", [H, H]), ("W2L", [H, H]), ("VA", [H, DX]),
        ("W3L", [H, DX]), ("W1E", [DX, H]), ("B3ROW", [1, DX]),
        ("VTCOL", [DX, 1]),
    ]:
        t_in[name] = nc.dram_tensor(name, shape, f, kind="ExternalInput").ap()
    XST = nc.dram_tensor("XST", [ns, DX, BC], f, kind="ExternalOutput").ap()
    VST = nc.dram_tensor("VST", [ns, DX, BC], f, kind="ExternalOutput").ap()
    LL = nc.dram_tensor("LL", [1, BC], f, kind="ExternalOutput").ap()

    Tanh = mybir.ActivationFunctionType.Tanh
    Copy = mybir.ActivationFunctionType.Copy
    MULT = mybir.AluOpType.mult
    ADD = mybir.AluOpType.add

    with tile.TileContext(nc) as tc:
        with (
            tc.tile_pool(name="const", bufs=1) as const,
            tc.tile_pool(name="state", bufs=PF + 3) as state_pool,
            tc.tile_pool(name="ltabp", bufs=PF + 3) as ltab_pool,
            tc.tile_pool(name="hp", bufs=3) as h_pool,
            tc.tile_pool(name="small", bufs=3) as small_pool,
            tc.tile_pool(name="ph1p", bufs=2, space="PSUM") as ph1p,
            tc.tile_pool(name="ph2p", bufs=1, space="PSUM") as ph2p,
            tc.tile_pool(name="pxp", bufs=2, space="PSUM") as pxp,
            tc.tile_pool(name="pqp", bufs=1, space="PSUM") as pqp,
            tc.tile_pool(name="pvp", bufs=1, space="PSUM") as pvp,
            tc.tile_pool(name="pllp", bufs=1, space="PSUM") as pllp,
        ):
            cst = {}
            for name in ["C64", "AW1", "WA", "W2L", "VA", "W3L", "W1E",
                         "B3ROW", "VTCOL", "B2COL", "B1T", "GCOLT", "KROW"]:
                ap = t_in[name]
                ctile = const.tile(list(ap.shape), f, name=f"c_{name}")
                nc.sync.dma_start(out=ctile, in_=ap)
                cst[name] = ctile
            ones_r = const.tile([1, BC], f, name="ones_r")
            nc.vector.memset(ones_r, 1.0)
            ones32 = const.tile([DX, 1], f, name="ones32")
            nc.vector.memset(ones32, 1.0)
            ll_ps = pllp.tile([1, BC], f, name="ll_ps")

            states = {}
            ltabs = {}
            qtabs = {}

            def new_state(k):
                st = state_pool.tile([64, BC], f, tag="state", name=f"st{k}")
                states[k] = st
                if k == 0:
                    nc.sync.dma_start(out=st[0:DX, :], in_=t_in["X0T"])
                if k < ns:
                    nc.sync.dma_start(out=st[DX:64, :], in_=t_in["DWT"][k])

            def load_ltab(k):
                lt = ltab_pool.tile([64, DX], f, tag="lt", name=f"lt{k}")
                qt = ltab_pool.tile([DX, DX], f, tag="qt", name=f"qt{k}")
                ltabs[k], qtabs[k] = lt, qt
                nc.sync.dma_start(out=lt, in_=t_in["LTAB"][k])
                nc.sync.dma_start(out=qt, in_=t_in["QTAB"][k])

            for k in range(PF):
                new_state(k)
                load_ltab(k)

            # bootstrap: xs2_0 and h1pre_0
            xs2 = small_pool.tile([DX, BC], f, tag="xs2", name="xs2_0")
            nc.vector.tensor_scalar_mul(out=xs2, in0=states[0][0:DX, :],
                                        scalar1=float(sc2[0]))
            ph1_cur = ph1p.tile([H, BC], f, tag="ph1", name="ph1_0")
            nc.tensor.matmul(ph1_cur, cst["W1E"], states[0][0:DX, :],
                             start=True, stop=True, skip_group_check=True)

            prev_u1 = prev_y = None
            for s in range(ns):
                st = states[s]
                # ll accumulation for step s-1 (off critical path)
                if prev_u1 is not None:
                    nc.tensor.matmul(ll_ps, ones32, prev_u1, start=(s == 1),
                                     stop=False, skip_group_check=True)
                    nc.tensor.matmul(ll_ps, cst["GCOLT"][:, s - 1:s], prev_y,
                                     start=False, stop=False,
                                     skip_group_check=True)
                # x' linear part, Q x, b3 (off-chain PE)
                px = pxp.tile([DX, BC], f, tag="px", name=f"px{s}")
                pq = pqp.tile([DX, BC], f, tag="pq", name=f"pq{s}")
                pv = pvp.tile([DX, BC], f, tag="pv", name=f"pv{s}")
                nc.tensor.matmul(px, ltabs[s], st[0:64, :],
                                 start=True, stop=False, skip_group_check=True)
                nc.tensor.matmul(px, cst["KROW"][0:1, s, :], ones_r,
                                 start=False, stop=False, skip_group_check=True)
                nc.tensor.matmul(pq, qtabs[s], st[0:DX, :],
                                 start=True, stop=True, skip_group_check=True)
                nc.tensor.matmul(pv, cst["B3ROW"], ones_r,
                                 start=True, stop=False, skip_group_check=True)
                # first-layer linear part for next step (off-chain PE)
                if s < ns - 1:
                    ph1_next = ph1p.tile([H, BC], f, tag="ph1",
                                         name=f"ph1_{s + 1}")
                    nc.tensor.matmul(ph1_next, cst["C64"], st[0:64, :],
                                     start=True, stop=False,
                                     skip_group_check=True)
                    nc.tensor.matmul(ph1_next, cst["AW1"], xs2,
                                     start=False, stop=False,
                                     skip_group_check=True)
                # y (off-chain DVE)
                y_t = small_pool.tile([DX, BC], f, tag="y", name=f"y{s}")
                nc.vector.tensor_scalar(y_t, st[0:DX, :], -1.0, cst["VTCOL"],
                                        MULT, ADD)
                # u1 = y * (Q x)  (off-chain DVE)
                u1 = small_pool.tile([DX, BC], f, tag="u1", name=f"u1_{s}")
                nc.vector.tensor_mul(u1, y_t, pq)
                # ---- critical chain ----
                h1 = h_pool.tile([H, BC], f, tag="h1", name=f"h1_{s}")
                nc.scalar.activation(h1, ph1_cur, Tanh,
                                     bias=cst["B1T"][:, s:s + 1], scale=1.0)
                ph2 = ph2p.tile([H, BC], f, tag="ph2", name=f"ph2_{s}")
                nc.tensor.matmul(ph2, cst["W2L"], h1, start=True, stop=True,
                                 skip_group_check=True)
                h2 = h_pool.tile([H, BC], f, tag="h2", name=f"h2_{s}")
                nc.scalar.activation(h2, ph2, Tanh, bias=cst["B2COL"],
                                     scale=1.0)
                if s < ns - 1:
                    nc.tensor.matmul(ph1_next, cst["WA"], h2, start=False,
                                     stop=True, skip_group_check=True)
                # ---- end chain ----
                nc.tensor.matmul(px, cst["VA"], h2, start=False, stop=True,
                                 skip_group_check=True)
                nc.tensor.matmul(pv, cst["W3L"], h2, start=False, stop=True,
                                 skip_group_check=True)
                if s + 1 not in states:
                    new_state(s + 1)
                st_next = states[s + 1]
                nc.scalar.activation(st_next[0:DX, :], px, Copy)
                if s < ns - 1:
                    xs2 = small_pool.tile([DX, BC], f, tag="xs2",
                                          name=f"xs2_{s + 1}")
                    nc.vector.tensor_scalar_mul(out=xs2,
                                                in0=st_next[0:DX, :],
                                                scalar1=float(sc2[s + 1]))
                vb = small_pool.tile([DX, BC], f, tag="vb", name=f"vb{s}")
                nc.vector.tensor_copy(vb, pv)
                nc.sync.dma_start(out=XST[s], in_=st_next[0:DX, :])
                nc.sync.dma_start(out=VST[s], in_=vb)
                # prefetch
                if s + PF <= ns and (s + PF) not in states:
                    new_state(s + PF)
                if s + PF < ns:
                    load_ltab(s + PF)
                prev_u1, prev_y = u1, y_t
                if s < ns - 1:
                    ph1_cur = ph1_next
            # final ll accumulation + store
            nc.tensor.matmul(ll_ps, ones32, prev_u1, start=(ns == 1),
                             stop=False, skip_group_check=True)
            nc.tensor.matmul(ll_ps, cst["GCOLT"][:, ns - 1:ns], prev_y,
                             start=False, stop=True, skip_group_check=True)
            ll_sb = const.tile([1, BC], f, name="ll_sb")
            nc.vector.tensor_copy(ll_sb, ll_ps)
            nc.sync.dma_start(out=LL, in_=ll_sb)
    nc.finalize()
    return nc


_prog_cache = {}


def _get_program(ns, sc2):
    key = (ns, sc2.tobytes())
    if key not in _prog_cache:
        _prog_cache[key] = _build_program(ns, sc2)
    return _prog_cache[key]


# ------------------------------------------------------------------- kernel
def kernel(x0, ts, dWs, W1, b1, W2, b2, W3, b3, Gmat, mu, v_target,
           _ns=NS, _trace=False):
    tb = _host_tables(ts, W1, b1, W2, b2, W3, b3, Gmat, mu, v_target, ns=_ns)
    nc = _get_program(_ns, tb["sc2"])

    dWt = (np.asarray(dWs, np.float32) * tb["sqdt"][:, None, None])
    x0 = np.asarray(x0, np.float32)

    shared = {k: tb[k] for k in ["LTAB", "QTAB", "KROW", "GCOLT", "B1T",
                                 "B2COL", "C64", "AW1", "WA", "W2L", "VA",
                                 "W3L", "W1E", "B3ROW", "VTCOL"]}
    in_maps = []
    for c in range(NCORES):
        lo, hi = c * BC, (c + 1) * BC
        m = dict(shared)
        m["X0T"] = np.ascontiguousarray(x0[lo:hi].T)
        m["DWT"] = np.ascontiguousarray(dWt[:, lo:hi, :].transpose(0, 2, 1))
        in_maps.append(m)

    res = bass_utils.run_bass_kernel_spmd(
        nc, in_maps, core_ids=list(range(NCORES)), trace=_trace)

    B = NCORES * BC
    xs = np.empty((_ns, B, DX), np.float32)
    vs = np.empty((_ns, B, DX), np.float32)
    ll = np.empty((B,), np.float32)
    for c in range(NCORES):
        lo, hi = c * BC, (c + 1) * BC
        r = res.results[c]
        xs[:, lo:hi, :] = r["XST"].transpose(0, 2, 1)
        vs[:, lo:hi, :] = r["VST"].transpose(0, 2, 1)
        ll[lo:hi] = r["LL"][0]
    if _trace:
        kernel.last_exec_time_ns = res.exec_time_ns
    return xs, vs, ll


# revision 5
# speedup vs baseline: 1.8137x; 1.8137x over previous
"""Trainium2 Bass kernel for the NeuralBridgeSDE sampling problem.

Reference scan step s (column vectors, a = G G^T, c_s = 1/(T-t_s+EPS)):
    y   = vt - x
    h1  = tanh(W1c x + b1 + t_s W1[0])            W1c = W1[1:].T
    h2  = tanh(W2c h1 + b2)
    v   = W3c h2 + b3
    x'  = M_s x + A h2 + G dWt_s + k_s            (dWt pre-scaled by sqrt(dt))
          M_s = (1-BETA dt) I - dt c_s a,  A = dt G W3.T,
          k_s = dt BETA mu + dt c_s a vt + dt G b3
    ll += y . (Q_s x + gcol_s)
          Q_s = -BETA dt c_s I + 0.5 dt c_s^2 a
          gcol_s = c_s dt BETA mu - 0.5 dt c_s^2 (a vt)

Device layout: feature-major [feat, batch] tiles, batch 1024 split 128 per
core across 8 cores (pure data parallel; the scan is sequential in time).

Per-core per-step state = [x; dWt; xs2] on partitions 0..95, with
xs2 = (-dt c_s) x.  Since M_s x = (1-BETA dt) x + a xs2, every per-step
matmul contracts the full 96-partition state (fp32 matmuls with K<=64 run
~1.7x slower than K>=96 on trn2, so everything is K-stacked/padded to 96+):

  MMX  [96x96]  state -> pxv: cols 0-31 x'-linear, 32-63 zero(v), 64-95 dup
  P4  [128x96]  h2    -> pxv += [A^T | W3 | A^T]  (v lands in cols 32-63)
  P3q  [96x32]  state -> pq = Q_s x   (per-step lhsT, zero-padded rows)
  P2   [96x128] state -> ph1_next  = W1c x' linear part
  P1  [128x128] h2    -> ph1_next += (W1c A) h2       [critical cycle]
  P6  [128x128] h1    -> ph2                           [critical cycle]

The critical cycle per step is only  P1 -> tanh -> P6 -> tanh.  The x/ll/v
updates ride PE/DVE/ACT slack: copy_x/xs2/y/u1/ll-accumulate on DVE, v-copy
on ACT.  ll is accumulated as a [32,B] running sum, reduced on host; b3 is
added to vs on host (it only shifts the v output, not the dynamics, which
absorbs G b3 into k_s).
"""

import numpy as np

import concourse.bacc as bacc
import concourse.tile as tile
from concourse import mybir
from concourse import bass_utils

BETA = 0.5
EPS = 1e-4
NS = 500
DX = 32
H = 128
NCORES = 8
BC = 128          # batch per core
PF = 4            # DMA prefetch depth (steps)
F32 = mybir.dt.float32


# ----------------------------------------------------------------- host math
def _host_tables(ts, W1, b1, W2, b2, W3, b3, Gmat, mu, v_target, ns=NS):
    ts = np.asarray(ts, np.float32)
    T = np.float32(ts[-1])
    dts = (ts[1:] - ts[:-1]).astype(np.float32)
    t_seq = np.empty(ns + 1, np.float32)
    t_seq[0] = ts[0]
    for s in range(ns):
        t_seq[s + 1] = np.float32(t_seq[s] + dts[s])
    D = ((T - t_seq[:ns]) + np.float32(EPS)).astype(np.float32)

    f64 = np.float64
    G = np.asarray(Gmat, f64)
    a = G @ G.T
    W1_ = np.asarray(W1, f64)
    W1e = W1_[1:, :]                       # [32,H]
    W3c = np.asarray(W3, f64).T
    mu64 = np.asarray(mu, f64)
    vt64 = np.asarray(v_target, f64)
    avt = a @ vt64
    Gb3 = G @ np.asarray(b3, f64)

    dt64 = dts.astype(f64)
    c = 1.0 / D.astype(f64)
    dt0 = dt64.mean()
    alpha = 1.0 - BETA * dt0

    I = np.eye(DX)
    Q = (-BETA * dt64 * c)[:, None, None] * I[None] + (
        0.5 * dt64 * c * c
    )[:, None, None] * a[None]                                      # [ns,32,32]
    QT96 = np.zeros((ns, 96, DX), np.float32)
    QT96[:, 0:DX, :] = Q.astype(np.float32)

    kvec = (dt64[:, None] * BETA * mu64[None]
            + (dt64 * c)[:, None] * avt[None]
            + dt64[:, None] * Gb3[None])                            # [ns,32]
    gcol = ((dt64 * c)[:, None] * BETA * mu64[None]
            - (0.5 * dt64 * c * c)[:, None] * avt[None])            # [ns,32]
    sc2 = (-(dt64 * c))                                             # [ns]

    W1c = W1e.T
    b1c = np.asarray(b1, f64)[None] + t_seq[:ns, None].astype(f64) * W1_[0][None]
    b1tot = b1c.copy()
    b1tot[1:] += kvec[: ns - 1] @ W1c.T                             # [ns,H]

    A = dt0 * (G @ W3c)                                             # [32,H]
    # x'-linear lhsT block over state=[x; dW; xs2]
    xlin = np.concatenate([alpha * I, G.T, a], axis=0)              # [96,32]
    MMX96 = np.zeros((96, 96), np.float32)
    MMX96[:, 0:DX] = xlin
    MMX96[:, 64:96] = xlin
    VP4 = np.concatenate([A.T, np.asarray(W3, f64), A.T], axis=1)   # [H,96]
    C96 = np.concatenate([alpha * W1e, G.T @ W1e, a @ W1e], axis=0) # [96,H]

    XKCOL = np.zeros((96, ns))
    XKCOL[64:96, : ns - 1] = (sc2[1:ns, None] * kvec[: ns - 1]).T

    return dict(
        QT96=np.ascontiguousarray(QT96),
        KCOL=np.ascontiguousarray(kvec.astype(np.float32).T),       # [32,ns]
        XKCOL=np.ascontiguousarray(XKCOL.astype(np.float32)),       # [96,ns]
        GCOL=np.ascontiguousarray(gcol.astype(np.float32).T),       # [32,ns]
        B1T=np.ascontiguousarray(b1tot.astype(np.float32).T),       # [H,ns]
        B2COL=np.ascontiguousarray(np.asarray(b2, np.float32)[:, None]),
        MMX96=np.ascontiguousarray(MMX96),
        VP4=np.ascontiguousarray(VP4.astype(np.float32)),
        C96=np.ascontiguousarray(C96.astype(np.float32)),
        WA=np.ascontiguousarray((A.T @ W1e).astype(np.float32)),    # [H,H]
        W2L=np.ascontiguousarray(np.asarray(W2, np.float32)),
        W1E=np.ascontiguousarray(np.asarray(W1, np.float32)[1:, :]),
        VTCOL=np.ascontiguousarray(np.asarray(v_target, np.float32)[:, None]),
        sc2=sc2.astype(np.float32),
        sqdt=np.sqrt(dts).astype(np.float32),
    )


# ------------------------------------------------------------ device program
def _build_program(ns, sc2):
    nc = bacc.Bacc("TRN2", target_bir_lowering=False, debug=False,
                   num_devices=NCORES)
    f = F32
    t_in = {}
    for name, shape in [
        ("X0T", [DX, BC]), ("X20", [DX, BC]), ("DWT", [ns, DX, BC]),
        ("QT96", [ns, 96, DX]), ("KCOL", [DX, ns]), ("XKCOL", [96, ns]),
        ("GCOL", [DX, ns]), ("B1T", [H, ns]), ("B2COL", [H, 1]),
        ("MMX96", [96, 96]), ("VP4", [H, 96]), ("C96", [96, H]),
        ("WA", [H, H]), ("W2L", [H, H]), ("W1E", [DX, H]), ("VTCOL", [DX, 1]),
    ]:
        t_in[name] = nc.dram_tensor(name, shape, f, kind="ExternalInput").ap()
    XST = nc.dram_tensor("XST", [ns, DX, BC], f, kind="ExternalOutput").ap()
    VST = nc.dram_tensor("VST", [ns, DX, BC], f, kind="ExternalOutput").ap()
    LL32 = nc.dram_tensor("LL32", [DX, BC], f, kind="ExternalOutput").ap()

    Tanh = mybir.ActivationFunctionType.Tanh
    Copy = mybir.ActivationFunctionType.Copy
    MULT = mybir.AluOpType.mult
    ADD = mybir.AluOpType.add

    with tile.TileContext(nc) as tc:
        with (
            tc.tile_pool(name="const", bufs=1) as const,
            tc.tile_pool(name="state", bufs=PF + 3) as state_pool,
            tc.tile_pool(name="qtabp", bufs=PF + 3) as qtab_pool,
            tc.tile_pool(name="hp", bufs=3) as h_pool,
            tc.tile_pool(name="small", bufs=3) as small_pool,
            tc.tile_pool(name="ph1p", bufs=2, space="PSUM") as ph1p,
            tc.tile_pool(name="ph2p", bufs=1, space="PSUM") as ph2p,
            tc.tile_pool(name="pxvp", bufs=2, space="PSUM") as pxvp,
            tc.tile_pool(name="pqp", bufs=2, space="PSUM") as pqp,
        ):
            cst = {}
            for name in ["MMX96", "VP4", "C96", "WA", "W2L", "W1E", "VTCOL",
                         "B2COL", "B1T", "KCOL", "XKCOL", "GCOL"]:
                ap = t_in[name]
                ctile = const.tile(list(ap.shape), f, name=f"c_{name}")
                nc.sync.dma_start(out=ctile, in_=ap)
                cst[name] = ctile
            acc = const.tile([DX, BC], f, name="acc")
            nc.vector.memset(acc, 0.0)

            states = {}
            qtabs = {}

            def new_state(k):
                st = state_pool.tile([96, BC], f, tag="state", name=f"st{k}")
                states[k] = st
                if k == 0:
                    nc.sync.dma_start(out=st[0:DX, :], in_=t_in["X0T"])
                    nc.sync.dma_start(out=st[64:96, :], in_=t_in["X20"])
                if k < ns:
                    nc.sync.dma_start(out=st[DX:64, :], in_=t_in["DWT"][k])

            def load_qtab(k):
                qt = qtab_pool.tile([96, DX], f, tag="qt", name=f"qt{k}")
                qtabs[k] = qt
                nc.sync.dma_start(out=qt, in_=t_in["QT96"][k])

            for k in range(PF):
                new_state(k)
                load_qtab(k)

            # bootstrap h1pre_0
            ph1_cur = ph1p.tile([H, BC], f, tag="ph1", name="ph1_0")
            nc.tensor.matmul(ph1_cur, cst["W1E"], states[0][0:DX, :],
                             start=True, stop=True, skip_group_check=True)

            for s in range(ns):
                st = states[s]
                # ---- off-chain PE on state_s
                pxv = pxvp.tile([96, BC], f, tag="pxv", name=f"pxv{s}")
                nc.tensor.matmul(pxv, cst["MMX96"], st[0:96, :],
                                 start=True, stop=False, skip_group_check=True)
                pq = pqp.tile([DX, BC], f, tag="pq", name=f"pq{s}")
                nc.tensor.matmul(pq, qtabs[s], st[0:96, :],
                                 start=True, stop=True, skip_group_check=True)
                if s < ns - 1:
                    ph1_next = ph1p.tile([H, BC], f, tag="ph1",
                                         name=f"ph1_{s + 1}")
                    nc.tensor.matmul(ph1_next, cst["C96"], st[0:96, :],
                                     start=True, stop=False,
                                     skip_group_check=True)
                # ---- off-chain DVE: y, ll accumulation
                y_t = small_pool.tile([DX, BC], f, tag="y", name=f"y{s}")
                nc.vector.tensor_scalar(y_t, st[0:DX, :], -1.0, cst["VTCOL"],
                                        MULT, ADD)
                u1 = small_pool.tile([DX, BC], f, tag="u1", name=f"u1_{s}")
                nc.vector.scalar_tensor_tensor(
                    u1, pq, cst["GCOL"][:, s:s + 1], y_t, ADD, MULT)
                nc.vector.tensor_tensor(acc, acc, u1, ADD)
                # ---- critical chain
                h1 = h_pool.tile([H, BC], f, tag="h1", name=f"h1_{s}")
                nc.scalar.activation(h1, ph1_cur, Tanh,
                                     bias=cst["B1T"][:, s:s + 1], scale=1.0)
                ph2 = ph2p.tile([H, BC], f, tag="ph2", name=f"ph2_{s}")
                nc.tensor.matmul(ph2, cst["W2L"], h1, start=True, stop=True,
                                 skip_group_check=True)
                h2 = h_pool.tile([H, BC], f, tag="h2", name=f"h2_{s}")
                nc.scalar.activation(h2, ph2, Tanh, bias=cst["B2COL"],
                                     scale=1.0)
                if s < ns - 1:
                    nc.tensor.matmul(ph1_next, cst["WA"], h2, start=False,
                                     stop=True, skip_group_check=True)
                # ---- end chain
                nc.tensor.matmul(pxv, cst["VP4"], h2, start=False, stop=True,
                                 skip_group_check=True)
                if s + 1 not in states:
                    new_state(s + 1)
                st_next = states[s + 1]
                nc.vector.tensor_scalar_add(out=st_next[0:DX, :],
                                            in0=pxv[0:DX, :],
                                            scalar1=cst["KCOL"][:, s:s + 1])
                if s < ns - 1:
                    nc.vector.tensor_scalar(
                        st_next[64:96, :], pxv[64:96, :], float(sc2[s + 1]),
                        cst["XKCOL"][64:96, s:s + 1], MULT, ADD)
                vb = small_pool.tile([64, BC], f, tag="vb", name=f"vb{s}")
                nc.scalar.activation(vb[DX:64, :], pxv[DX:64, :], Copy)
                nc.sync.dma_start(out=XST[s], in_=st_next[0:DX, :])
                nc.sync.dma_start(out=VST[s], in_=vb[DX:64, :])
                # ---- prefetch
                if s + PF <= ns and (s + PF) not in states:
                    new_state(s + PF)
                if s + PF < ns:
                    load_qtab(s + PF)
                if s < ns - 1:
                    ph1_cur = ph1_next
            nc.sync.dma_start(out=LL32, in_=acc)
    nc.finalize()
    return nc


_prog_cache = {}


def _get_program(ns, sc2):
    key = (ns, sc2.tobytes())
    if key not in _prog_cache:
        _prog_cache[key] = _build_program(ns, sc2)
    return _prog_cache[key]


# ------------------------------------------------------------------- kernel
def kernel(x0, ts, dWs, W1, b1, W2, b2, W3, b3, Gmat, mu, v_target,
           _ns=NS, _trace=False):
    tb = _host_tables(ts, W1, b1, W2, b2, W3, b3, Gmat, mu, v_target, ns=_ns)
    nc = _get_program(_ns, tb["sc2"])

    dWt = (np.asarray(dWs, np.float32) * tb["sqdt"][:, None, None])
    x0 = np.asarray(x0, np.float32)

    shared = {k: tb[k] for k in ["QT96", "KCOL", "XKCOL", "GCOL", "B1T",
                                 "B2COL", "MMX96", "VP4", "C96", "WA", "W2L",
                                 "W1E", "VTCOL"]}
    in_maps = []
    for c in range(NCORES):
        lo, hi = c * BC, (c + 1) * BC
        m = dict(shared)
        x0t = np.ascontiguousarray(x0[lo:hi].T)
        m["X0T"] = x0t
        m["X20"] = np.ascontiguousarray(tb["sc2"][0] * x0t)
        m["DWT"] = np.ascontiguousarray(dWt[:, lo:hi, :].transpose(0, 2, 1))
        in_maps.append(m)

    res = bass_utils.run_bass_kernel_spmd(
        nc, in_maps, core_ids=list(range(NCORES)), trace=_trace)

    B = NCORES * BC
    b3f = np.asarray(b3, np.float32)
    xs = np.empty((_ns, B, DX), np.float32)
    vs = np.empty((_ns, B, DX), np.float32)
    ll = np.empty((B,), np.float32)
    for c in range(NCORES):
        lo, hi = c * BC, (c + 1) * BC
        r = res.results[c]
        xs[:, lo:hi, :] = r["XST"].transpose(0, 2, 1)
        vs[:, lo:hi, :] = r["VST"].transpose(0, 2, 1) + b3f[None, None, :]
        ll[lo:hi] = r["LL32"].sum(axis=0, dtype=np.float32)
    if _trace:
        kernel.last_exec_time_ns = res.exec_time_ns
    return xs, vs, ll


# revision 6
# speedup vs baseline: 1.9691x; 1.0857x over previous
"""Trainium2 Bass kernel for the NeuralBridgeSDE sampling problem.

Reference scan step s (column vectors, a = G G^T, c_s = 1/(T-t_s+EPS)):
    y   = vt - x
    h1  = tanh(W1c x + b1 + t_s W1[0])            W1c = W1[1:].T
    h2  = tanh(W2c h1 + b2)
    v   = W3c h2 + b3
    x'  = M_s x + A h2 + G dWt_s + k_s            (dWt pre-scaled by sqrt(dt))
          M_s = (1-BETA dt) I - dt c_s a,  A = dt G W3.T,
          k_s = dt BETA mu + dt c_s a vt + dt G b3
    ll += y . (Q_s x + gcol_s)
          Q_s = -BETA dt c_s I + 0.5 dt c_s^2 a
          gcol_s = c_s dt BETA mu - 0.5 dt c_s^2 (a vt)

Device layout: feature-major [feat, batch] tiles, batch 1024 split 128 per
core across 8 cores (pure data parallel; the scan is sequential in time).

Per-core per-step state = [x; dWt; xs2] on partitions 0..95, with
xs2 = (-dt c_s) x.  Since M_s x = (1-BETA dt) x + a xs2, every per-step
matmul contracts the full 96-partition state (fp32 matmuls with K<=64 run
~1.7x slower than K>=96 on trn2, so everything is K-stacked/padded to 96+):

  MMX  [96x96]  state -> pxv: cols 0-31 x'-linear, 32-63 zero(v), 64-95 dup
  P4  [128x96]  h2    -> pxv += [A^T | W3 | A^T]  (v lands in cols 32-63)
  P3q  [96x32]  state -> pq = Q_s x   (per-step lhsT, zero-padded rows)
  P2   [96x128] state -> ph1_next  = W1c x' linear part
  P1  [128x128] h2    -> ph1_next += (W1c A) h2       [critical cycle]
  P6  [128x128] h1    -> ph2                           [critical cycle]

The critical cycle per step is only  P1 -> tanh -> P6 -> tanh.  The x/ll/v
updates ride PE/DVE/ACT slack: copy_x/xs2/y/u1/ll-accumulate on DVE, v-copy
on ACT.  ll is accumulated as a [32,B] running sum, reduced on host; b3 is
added to vs on host (it only shifts the v output, not the dynamics, which
absorbs G b3 into k_s).
"""

import numpy as np

import concourse.bacc as bacc
import concourse.tile as tile
from concourse import mybir
from concourse import bass_utils

BETA = 0.5
EPS = 1e-4
NS = 500
DX = 32
H = 128
NCORES = 8
BC = 128          # batch per core
PF = 4            # DMA prefetch depth (steps)
F32 = mybir.dt.float32


# ----------------------------------------------------------------- host math
def _host_tables(ts, W1, b1, W2, b2, W3, b3, Gmat, mu, v_target, ns=NS):
    ts = np.asarray(ts, np.float32)
    T = np.float32(ts[-1])
    dts = (ts[1:] - ts[:-1]).astype(np.float32)
    t_seq = np.empty(ns + 1, np.float32)
    t_seq[0] = ts[0]
    for s in range(ns):
        t_seq[s + 1] = np.float32(t_seq[s] + dts[s])
    D = ((T - t_seq[:ns]) + np.float32(EPS)).astype(np.float32)

    f64 = np.float64
    G = np.asarray(Gmat, f64)
    a = G @ G.T
    W1_ = np.asarray(W1, f64)
    W1e = W1_[1:, :]                       # [32,H]
    W3c = np.asarray(W3, f64).T
    mu64 = np.asarray(mu, f64)
    vt64 = np.asarray(v_target, f64)
    avt = a @ vt64
    Gb3 = G @ np.asarray(b3, f64)

    dt64 = dts.astype(f64)
    c = 1.0 / D.astype(f64)
    dt0 = dt64.mean()
    alpha = 1.0 - BETA * dt0

    I = np.eye(DX)
    Q = (-BETA * dt64 * c)[:, None, None] * I[None] + (
        0.5 * dt64 * c * c
    )[:, None, None] * a[None]                                      # [ns,32,32]

    kvec = (dt64[:, None] * BETA * mu64[None]
            + (dt64 * c)[:, None] * avt[None]
            + dt64[:, None] * Gb3[None])                            # [ns,32]
    gcol = ((dt64 * c)[:, None] * BETA * mu64[None]
            - (0.5 * dt64 * c * c)[:, None] * avt[None])            # [ns,32]
    sc2 = (-(dt64 * c))                                             # [ns]

    W1c = W1e.T
    b1c = np.asarray(b1, f64)[None] + t_seq[:ns, None].astype(f64) * W1_[0][None]
    b1tot = b1c.copy()
    b1tot[1:] += kvec[: ns - 1] @ W1c.T                             # [ns,H]

    A = dt0 * (G @ W3c)                                             # [32,H]
    # per-step main-state lhsT: [x'lin | Q_s x | x'lin dup | 0] over
    # state=[x; dW; xs2]; v comes from P4 into cols 96-127.
    xlin = np.concatenate([alpha * I, G.T, a], axis=0)              # [96,32]
    QMX = np.zeros((ns, 96, 128), np.float32)
    QMX[:, :, 0:DX] = xlin[None]
    QMX[:, 0:DX, DX:64] = Q.astype(np.float32)
    QMX[:, :, 64:96] = xlin[None]
    VP4 = np.zeros((H, 128))
    VP4[:, 0:DX] = A.T
    VP4[:, 64:96] = A.T
    VP4[:, 96:128] = np.asarray(W3, f64)
    C96 = np.concatenate([alpha * W1e, G.T @ W1e, a @ W1e], axis=0) # [96,H]

    XKCOL = np.zeros((96, ns))
    XKCOL[64:96, : ns - 1] = (sc2[1:ns, None] * kvec[: ns - 1]).T

    return dict(
        QMX=np.ascontiguousarray(QMX),
        KCOL=np.ascontiguousarray(kvec.astype(np.float32).T),       # [32,ns]
        XKCOL=np.ascontiguousarray(XKCOL.astype(np.float32)),       # [96,ns]
        GCOL=np.ascontiguousarray(gcol.astype(np.float32).T),       # [32,ns]
        B1T=np.ascontiguousarray(b1tot.astype(np.float32).T),       # [H,ns]
        B2COL=np.ascontiguousarray(np.asarray(b2, np.float32)[:, None]),
        VP4=np.ascontiguousarray(VP4.astype(np.float32)),
        C96=np.ascontiguousarray(C96.astype(np.float32)),
        WA=np.ascontiguousarray((A.T @ W1e).astype(np.float32)),    # [H,H]
        W2L=np.ascontiguousarray(np.asarray(W2, np.float32)),
        W1E=np.ascontiguousarray(np.asarray(W1, np.float32)[1:, :]),
        VTCOL=np.ascontiguousarray(np.asarray(v_target, np.float32)[:, None]),
        sc2=sc2.astype(np.float32),
        sqdt=np.sqrt(dts).astype(np.float32),
    )


# ------------------------------------------------------------ device program
def _build_program(ns, sc2):
    nc = bacc.Bacc("TRN2", target_bir_lowering=False, debug=False,
                   num_devices=NCORES)
    f = F32
    t_in = {}
    for name, shape in [
        ("X0T", [DX, BC]), ("X20", [DX, BC]), ("DWT", [ns, DX, BC]),
        ("QMX", [ns, 96, 128]), ("KCOL", [DX, ns]), ("XKCOL", [96, ns]),
        ("GCOL", [DX, ns]), ("B1T", [H, ns]), ("B2COL", [H, 1]),
        ("VP4", [H, 128]), ("C96", [96, H]),
        ("WA", [H, H]), ("W2L", [H, H]), ("W1E", [DX, H]), ("VTCOL", [DX, 1]),
    ]:
        t_in[name] = nc.dram_tensor(name, shape, f, kind="ExternalInput").ap()
    XST = nc.dram_tensor("XST", [ns, DX, BC], f, kind="ExternalOutput").ap()
    VST = nc.dram_tensor("VST", [ns, DX, BC], f, kind="ExternalOutput").ap()
    LL32 = nc.dram_tensor("LL32", [DX, BC], f, kind="ExternalOutput").ap()

    Tanh = mybir.ActivationFunctionType.Tanh
    Copy = mybir.ActivationFunctionType.Copy
    MULT = mybir.AluOpType.mult
    ADD = mybir.AluOpType.add

    with tile.TileContext(nc) as tc:
        with (
            tc.tile_pool(name="const", bufs=1) as const,
            tc.tile_pool(name="state", bufs=PF + 3) as state_pool,
            tc.tile_pool(name="qtabp", bufs=PF + 3) as qtab_pool,
            tc.tile_pool(name="hp", bufs=3) as h_pool,
            tc.tile_pool(name="small", bufs=3) as small_pool,
            tc.tile_pool(name="ph1p", bufs=2, space="PSUM") as ph1p,
            tc.tile_pool(name="ph2p", bufs=1, space="PSUM") as ph2p,
            tc.tile_pool(name="pxvp", bufs=2, space="PSUM") as pxvp,
        ):
            cst = {}
            for name in ["VP4", "C96", "WA", "W2L", "W1E", "VTCOL",
                         "B2COL", "B1T", "KCOL", "XKCOL", "GCOL"]:
                ap = t_in[name]
                ctile = const.tile(list(ap.shape), f, name=f"c_{name}")
                nc.sync.dma_start(out=ctile, in_=ap)
                cst[name] = ctile
            acc = const.tile([DX, BC], f, name="acc")
            nc.vector.memset(acc, 0.0)

            states = {}
            qtabs = {}

            def new_state(k):
                st = state_pool.tile([96, BC], f, tag="state", name=f"st{k}")
                states[k] = st
                if k == 0:
                    nc.sync.dma_start(out=st[0:DX, :], in_=t_in["X0T"])
                    nc.sync.dma_start(out=st[64:96, :], in_=t_in["X20"])
                if k < ns:
                    nc.sync.dma_start(out=st[DX:64, :], in_=t_in["DWT"][k])

            def load_qtab(k):
                qt = qtab_pool.tile([96, 128], f, tag="qt", name=f"qt{k}")
                qtabs[k] = qt
                nc.sync.dma_start(out=qt, in_=t_in["QMX"][k])

            for k in range(PF):
                new_state(k)
                load_qtab(k)

            # bootstrap h1pre_0
            ph1_cur = ph1p.tile([H, BC], f, tag="ph1", name="ph1_0")
            nc.tensor.matmul(ph1_cur, cst["W1E"], states[0][0:DX, :],
                             start=True, stop=True, skip_group_check=True)

            for s in range(ns):
                st = states[s]
                # ---- off-chain PE on state_s
                pxv = pxvp.tile([128, BC], f, tag="pxv", name=f"pxv{s}")
                nc.tensor.matmul(pxv, qtabs[s], st[0:96, :],
                                 start=True, stop=False, skip_group_check=True)
                if s < ns - 1:
                    ph1_next = ph1p.tile([H, BC], f, tag="ph1",
                                         name=f"ph1_{s + 1}")
                    nc.tensor.matmul(ph1_next, cst["C96"], st[0:96, :],
                                     start=True, stop=False,
                                     skip_group_check=True)
                # ---- off-chain DVE: y, ll accumulation
                y_t = small_pool.tile([DX, BC], f, tag="y", name=f"y{s}")
                nc.vector.tensor_scalar(y_t, st[0:DX, :], -1.0, cst["VTCOL"],
                                        MULT, ADD)
                u1 = small_pool.tile([DX, BC], f, tag="u1", name=f"u1_{s}")
                nc.vector.scalar_tensor_tensor(
                    u1, pxv[DX:64, :], cst["GCOL"][:, s:s + 1], y_t, ADD, MULT)
                nc.vector.tensor_tensor(acc, acc, u1, ADD)
                # ---- critical chain
                h1 = h_pool.tile([H, BC], f, tag="h1", name=f"h1_{s}")
                nc.scalar.activation(h1, ph1_cur, Tanh,
                                     bias=cst["B1T"][:, s:s + 1], scale=1.0)
                ph2 = ph2p.tile([H, BC], f, tag="ph2", name=f"ph2_{s}")
                nc.tensor.matmul(ph2, cst["W2L"], h1, start=True, stop=True,
                                 skip_group_check=True)
                h2 = h_pool.tile([H, BC], f, tag="h2", name=f"h2_{s}")
                nc.scalar.activation(h2, ph2, Tanh, bias=cst["B2COL"],
                                     scale=1.0)
                if s < ns - 1:
                    nc.tensor.matmul(ph1_next, cst["WA"], h2, start=False,
                                     stop=True, skip_group_check=True)
                # ---- end chain
                nc.tensor.matmul(pxv, cst["VP4"], h2, start=False, stop=True,
                                 skip_group_check=True)
                if s + 1 not in states:
                    new_state(s + 1)
                st_next = states[s + 1]
                nc.vector.tensor_scalar_add(out=st_next[0:DX, :],
                                            in0=pxv[0:DX, :],
                                            scalar1=cst["KCOL"][:, s:s + 1])
                if s < ns - 1:
                    nc.vector.tensor_scalar(
                        st_next[64:96, :], pxv[64:96, :], float(sc2[s + 1]),
                        cst["XKCOL"][64:96, s:s + 1], MULT, ADD)
                vb = small_pool.tile([128, BC], f, tag="vb", name=f"vb{s}")
                nc.scalar.activation(vb[96:128, :], pxv[96:128, :], Copy)
                nc.sync.dma_start(out=XST[s], in_=st_next[0:DX, :])
                nc.sync.dma_start(out=VST[s], in_=vb[96:128, :])
                # ---- prefetch
                if s + PF <= ns and (s + PF) not in states:
                    new_state(s + PF)
                if s + PF < ns:
                    load_qtab(s + PF)
                if s < ns - 1:
                    ph1_cur = ph1_next
            nc.sync.dma_start(out=LL32, in_=acc)
    nc.finalize()
    return nc


_prog_cache = {}


def _get_program(ns, sc2):
    key = (ns, sc2.tobytes())
    if key not in _prog_cache:
        _prog_cache[key] = _build_program(ns, sc2)
    return _prog_cache[key]


# ------------------------------------------------------------------- kernel
def kernel(x0, ts, dWs, W1, b1, W2, b2, W3, b3, Gmat, mu, v_target,
           _ns=NS, _trace=False):
    tb = _host_tables(ts, W1, b1, W2, b2, W3, b3, Gmat, mu, v_target, ns=_ns)
    nc = _get_program(_ns, tb["sc2"])

    dWt = (np.asarray(dWs, np.float32) * tb["sqdt"][:, None, None])
    x0 = np.asarray(x0, np.float32)

    shared = {k: tb[k] for k in ["QMX", "KCOL", "XKCOL", "GCOL", "B1T",
                                 "B2COL", "VP4", "C96", "WA", "W2L",
                                 "W1E", "VTCOL"]}
    in_maps = []
    for c in range(NCORES):
        lo, hi = c * BC, (c + 1) * BC
        m = dict(shared)
        x0t = np.ascontiguousarray(x0[lo:hi].T)
        m["X0T"] = x0t
        m["X20"] = np.ascontiguousarray(tb["sc2"][0] * x0t)
        m["DWT"] = np.ascontiguousarray(dWt[:, lo:hi, :].transpose(0, 2, 1))
        in_maps.append(m)

    res = bass_utils.run_bass_kernel_spmd(
        nc, in_maps, core_ids=list(range(NCORES)), trace=_trace)

    B = NCORES * BC
    b3f = np.asarray(b3, np.float32)
    xs = np.empty((_ns, B, DX), np.float32)
    vs = np.empty((_ns, B, DX), np.float32)
    ll = np.empty((B,), np.float32)
    for c in range(NCORES):
        lo, hi = c * BC, (c + 1) * BC
        r = res.results[c]
        xs[:, lo:hi, :] = r["XST"].transpose(0, 2, 1)
        vs[:, lo:hi, :] = r["VST"].transpose(0, 2, 1) + b3f[None, None, :]
        ll[lo:hi] = r["LL32"].sum(axis=0, dtype=np.float32)
    if _trace:
        kernel.last_exec_time_ns = res.exec_time_ns
    return xs, vs, ll
